# revision 29
# baseline (speedup 1.0000x reference)
"""Gated GQA self-attention with KV cache, tensor-parallel over heads on 8
Trainium2 NeuronCores.

Reference computation (fp32):
    q = rms_norm((x @ w_q.T).reshape(B,L,H,HD))      # per-head rms over HD
    k = rms_norm((x @ w_k.T).reshape(B,L,HKV,HD))
    v = (x @ w_v.T).reshape(B,L,HKV,HD)
    k_t/v_t = concat(cache, new) over seq -> [B,HKV,S,HD]
    o = softmax(q @ k_t.T / sqrt(HD)) @ v_t          # full (non-causal)
    o *= sigmoid(x[..., :16] @ w_gate.T)             # per-head gate
    y = o.reshape(B,L,D) @ w_out.T

Sharding: core c owns q heads {2c, 2c+1} and kv group g=c//2.  Each core
computes its heads' attention plus the partial out-projection
y_c = o_c @ w_out[:, cols_c].T; the host sums the 8 partials.

v3 design notes:
  * bf16 matmul operands everywhere; fp32 only in PSUM and the small
    normalization rows (~6e-3 max-rel total, gate is 2e-2).
  * Attention is one flat software pipeline over all 64 (block, s-pair)
    steps: scores for pair g+2 issue before P@V of pair g, crossing block
    boundaries, so the PE never drains and the HAM clock stays at 2.4GHz.
  * exp on [128,2,512] double-chunks (amortizes ACT fixed cost); softmax
    denominator via a DVE add-tree + one ones-matmul per block.
  * gate/denominator factors: rank-1 PE broadcast multiplied into the raw
    attention output once per block; the two heads then share one PSUM
    accumulation in the out-projection.  All row reciprocals use the ~5x
    faster reciprocal_approx_fast (18-bit accurate).
  * Out-projection emitted in 8-matmul li-bursts at block boundaries
    (deferred ~a block from its unit) - spreads y DMA and PE load.
  * Startup DMAs split k-chunk-wise so the first matmul waits on ~128KB,
    not megabytes.
"""

from contextlib import ExitStack

import ml_dtypes
import numpy as np

import concourse.bass as bass  # noqa: F401
import concourse.tile as tile
from concourse import bacc, mybir
from concourse.bass_utils import run_bass_kernel_spmd

F32R = mybir.dt.float32r
F32 = mybir.dt.float32
BF16 = mybir.dt.bfloat16
AF = mybir.ActivationFunctionType

B, L, D = 2, 1024, 2048
H, HKV, HD = 16, 4, 128
CACHE = 1024
BL = B * L                  # 2048
S = CACHE + L               # 2048
NCORES = 8
QH = H // NCORES            # 2 q heads per core
JC = QH * HD                # 256 out-proj contraction cols per core
EPS = 1e-6
NLP = BL // 512             # 4 l-chunks in phase 1
ND = D // 128               # 16 contraction chunks
NS = S // 128               # 16 s-chunks per batch
NSC = CACHE // 128          # 8 cached s chunks
NP = NS // 2                # 8 s-chunk pairs per block

_CACHED_NC = None


def _build_core_program():
    nc = bacc.Bacc("TRN2", target_bir_lowering=False, debug=False)

    xt = nc.dram_tensor("xt", [128, NLP, ND, 512], BF16, kind="ExternalInput").ap()
    wqkv = nc.dram_tensor("wqkv", [128, ND, 4 * HD], BF16, kind="ExternalInput").ap()
    wo = nc.dram_tensor("wo", [128, QH, D], BF16, kind="ExternalInput").ap()
    wg = nc.dram_tensor("wg", [H, QH], BF16, kind="ExternalInput").ap()
    ckt = nc.dram_tensor("ckt", [B, HD, CACHE], BF16, kind="ExternalInput").ap()
    cv = nc.dram_tensor("cv", [B, 128, NSC, HD], BF16, kind="ExternalInput").ap()
    identb_in = nc.dram_tensor("identb", [128, 128], BF16, kind="ExternalInput").ap()
    onesb_in = nc.dram_tensor("onesb", [128, 1], BF16, kind="ExternalInput").ap()
    onesr_in = nc.dram_tensor("onesr", [1, 128], F32R, kind="ExternalInput").ap()
    y = nc.dram_tensor("y", [BL, D], BF16, kind="ExternalOutput").ap()

    with tile.TileContext(nc) as tc, ExitStack() as ctx:
        singles = ctx.enter_context(tc.tile_pool(name="singles", bufs=1))
        xtp = ctx.enter_context(tc.tile_pool(name="xtp", bufs=2))
        sqp = ctx.enter_context(tc.tile_pool(name="sqp", bufs=2))
        exp_ = ctx.enter_context(tc.tile_pool(name="exp", bufs=6))
        accp = ctx.enter_context(tc.tile_pool(name="accp", bufs=2))
        fdnp = ctx.enter_context(tc.tile_pool(name="fdnp", bufs=2))
        ysbp = ctx.enter_context(tc.tile_pool(name="ysbp", bufs=2))

        psBig = ctx.enter_context(tc.tile_pool(name="psBig", bufs=2, space="PSUM"))
        psO = ctx.enter_context(tc.tile_pool(name="psO", bufs=2, space="PSUM"))
        psM = ctx.enter_context(tc.tile_pool(name="psM", bufs=1, space="PSUM"))
        ypP = ctx.enter_context(tc.tile_pool(name="ypP", bufs=1, space="PSUM"))

        lowp = nc.allow_low_precision(reason="bf16/f32r rounding is intended")
        ctx.enter_context(lowp)

        identb = singles.tile([128, 128], BF16)
        nc.scalar.dma_start(out=identb, in_=identb_in)
        onesb = singles.tile([128, 1], BF16)
        nc.scalar.dma_start(out=onesb, in_=onesb_in)
        onesr = singles.tile([1, 128], F32R)
        nc.scalar.dma_start(out=onesr, in_=onesr_in)
        wg_sb = singles.tile([H, QH], BF16)
        nc.scalar.dma_start(out=wg_sb, in_=wg)

        bias_q = singles.tile([1, 1], F32)
        nc.vector.memset(bias_q, HD * EPS)
        bias_k = singles.tile([1, 1], F32)
        nc.vector.memset(bias_k, EPS)

        # wqkv: k-chunk-major fine splits so the first LDWEIGHTS waits ~128KB.
        # The k=0 chunks ride the scalar queue (already warm from the consts)
        # so the first matmul isn't gated on sync-queue spin-up.
        wqkv_sb = singles.tile([128, ND, 4 * HD], BF16)
        nc.scalar.dma_start(out=wqkv_sb[:, 0:1, :], in_=wqkv[:, 0:1, :])
        for ka, kb in ((1, 4), (4, 10), (10, 16)):
            nc.sync.dma_start(
                out=wqkv_sb[:, ka:kb, :], in_=wqkv[:, ka:kb, :]
            )
        wo_sb = singles.tile([128, QH, D], BF16)

        qkvt = singles.tile([128, 4, BL], BF16)       # jc: qh0, qh1, k, v
        otg = singles.tile([128, B, QH, 2, 512], BF16)
        glog = [
            singles.tile([1, BL], F32R, tag=f"glog{h}", name=f"glog{h}")
            for h in range(QH)
        ]
        grow = [
            singles.tile([1, BL], F32R, tag=f"grow{h}", name=f"grow{h}")
            for h in range(QH)
        ]
        rows = [
            singles.tile([1, BL], F32R, tag=f"row{i}", name=f"row{i}")
            for i in range(3)
        ]
        cache_tiles = {}
        vnew = {}

        def emit_prefetch():
            nc.scalar.dma_start(out=wo_sb, in_=wo)
            for b in range(B):
                ck_sb = singles.tile([128, CACHE], BF16, tag=f"ck{b}", name=f"ck{b}")
                nc.scalar.dma_start(out=ck_sb, in_=ckt[b])
                cv_sb = singles.tile([128, NSC, HD], BF16, tag=f"cv{b}", name=f"cv{b}")
                nc.scalar.dma_start(out=cv_sb, in_=cv[b])
                cache_tiles[b] = (ck_sb, cv_sb)
                vnew[b] = singles.tile(
                    [128, NSC, HD], BF16, tag=f"vn{b}", name=f"vn{b}"
                )

        from concourse.dve_ops import (
            RECIP_APPROX_FAST_CONSTS as _RC,
            RECIPROCAL_APPROX_FAST as _RF,
        )

        def recip_fast(out_f32r, in_f32):
            """~18-bit 1/x on DVE, writing an f32r-typed row (the wrapper
            insists on fp32 both sides; f32r shares the fp32 bit layout)."""
            nc.vector._custom_dve(
                _RF, out=out_f32r, in0=in_f32,
                s0=_RC["s0"], s1=_RC["s1"], imm2=_RC["imm2"],
            )

        pe_defer = []  # (key, fn): deferred small PE ops, flushed between groups

        def flush_one_defer():
            if pe_defer:
                pe_defer.pop(0)[1]()

        def flush_key(key):
            """Run a specific deferred op now (prerequisite ordering)."""
            for idx, (k, fn) in enumerate(pe_defer):
                if k == key:
                    pe_defer.pop(idx)
                    fn()
                    return

        def apply_norm(lc, jc):
            """reciprocal of one rms row chunk + column normalize (deferred)."""
            flush_key(("ssq", lc, jc))  # the sqrt must be emitted before us
            sl = slice(lc * 512, lc * 512 + 512)
            recip_fast(rows[jc][:, sl], rows[jc][:, sl].bitcast(F32))

            def bc_mul(jc=jc, sl=sl, lc=lc):
                bc = psM.tile([128, 512], F32, tag="m", name=f"bc{jc}_{lc}")
                nc.tensor.matmul(bc, onesr, rows[jc][:, sl], start=True, stop=True)
                nc.vector.tensor_mul(qkvt[:, jc, sl], qkvt[:, jc, sl], bc)

            pe_defer.append((("bc", lc, jc), bc_mul))

        # ---- phase 1: projections -------------------------------------
        for lc in range(NLP):
            sl = slice(lc * 512, lc * 512 + 512)
            xtile = xtp.tile([128, ND, 512], BF16, tag="xt")
            if lc == 0:
                nc.scalar.dma_start(out=xtile[:, 0:1, :], in_=xt[:, 0, 0:1, :])
                for ka, kb in ((1, 4), (4, 10), (10, 16)):
                    nc.sync.dma_start(
                        out=xtile[:, ka:kb, :], in_=xt[:, 0, ka:kb, :]
                    )
                emit_prefetch()
            else:
                nc.sync.dma_start(out=xtile, in_=xt[:, lc])
            # gate logit rows for this chunk
            for h in range(QH):
                gps = psM.tile([1, 512], F32, tag="m", name=f"gps{lc}{h}")
                nc.tensor.matmul(
                    gps, wg_sb[:, h : h + 1], xtile[0:H, 0, :],
                    start=True, stop=True,
                )
                nc.vector.tensor_copy(glog[h][:, sl], gps)
            if lc >= 1:
                apply_norm(lc - 1, 2)  # k first: unblocks attention earliest
            for pi, pair in enumerate(((3, 2), (0, 1))):  # v,k first
                pp = psBig.tile([128, 2, 512], F32, tag="big", name=f"pp{lc}_{pi}")
                for j, jc in enumerate(pair):
                    for kk in range(ND):
                        nc.tensor.matmul(
                            pp[:, j, :],
                            wqkv_sb[:, kk, jc * 128 : jc * 128 + 128],
                            xtile[:, kk, :],
                            start=(kk == 0),
                            stop=(kk == ND - 1),
                        )
                    flush_one_defer()
                for j, jc in enumerate(pair):
                    if jc < QH:
                        nc.scalar.copy(qkvt[:, jc, sl], pp[:, j, :])
                    else:
                        nc.vector.tensor_copy(qkvt[:, jc, sl], pp[:, j, :])
                    if jc < 3:  # q0, q1, k: rms stats
                        sq = sqp.tile([128, 512], BF16, tag="sq")
                        nc.vector.tensor_mul(sq, qkvt[:, jc, sl], qkvt[:, jc, sl])

                        def ssq_mm(jc=jc, sl=sl, sq=sq, lc=lc):
                            ssq = psM.tile(
                                [1, 512], F32, tag="m", name=f"ssq{lc}{jc}"
                            )
                            nc.tensor.matmul(ssq, onesb, sq, start=True, stop=True)
                            scale, bias = (
                                (1.0, bias_q) if jc < QH else (1.0 / HD, bias_k)
                            )
                            nc.scalar.activation(
                                rows[jc][:, sl], ssq, AF.Sqrt,
                                bias=bias[:], scale=scale,
                            )

                        pe_defer.append((("ssq", lc, jc), ssq_mm))
                    flush_one_defer()
                if lc >= 1 and pi == 0:
                    apply_norm(lc - 1, 0)
            if lc >= 1:
                apply_norm(lc - 1, 1)

        # tail rms chunks: reciprocals now, broadcast-applies deferred into
        # the early attention pair stream (their consumers live in b=1,
        # ~40us away) so the PE never pauses between phases
        apply_norm(NLP - 1, 2)
        apply_norm(NLP - 1, 0)
        apply_norm(NLP - 1, 1)
        # batched gate sigmoids (keeps the ACT table on EXP afterwards)
        for h in range(QH):
            nc.scalar.activation(grow[h], glog[h], AF.Sigmoid)

        # ---- phase 2 + 3: flat pipeline -------------------------------
        blocks = [(b, lc2, h) for b in range(B) for lc2 in range(2) for h in range(QH)]
        NB = len(blocks)
        st = {bi: dict() for bi in range(NB)}
        pend_den = []   # den-chain emitters, flushed one block later
        pend_p3 = []    # (b, lc2, li) out-proj bursts

        def q_of(bi):
            b, lc2, h = blocks[bi]
            off = b * L + lc2 * 512
            return qkvt[:, h, off : off + 512]

        def emit_sp(bi, i):
            b, lc2, h = blocks[bi]
            boff = b * L
            ck_sb, _ = cache_tiles[b]
            sp = psBig.tile([128, 2, 512], F32, tag="big", name=f"sp{bi}_{i}")
            for j in range(2):
                sc = 2 * i + j
                if sc < NSC:
                    kT = ck_sb[:, sc * 128 : sc * 128 + 128]
                else:
                    jj = boff + (sc - NSC) * 128
                    kT = qkvt[:, 2, jj : jj + 128]
                nc.tensor.matmul(sp[:, j, :], kT, q_of(bi), start=True, stop=True)
            st[bi].setdefault("sps", {})[i] = sp

        def block_end(bi):
            b, lc2, h = blocks[bi]
            s = st[bi]
            nc.vector.tensor_copy(otg[:, b, h, lc2, :], s["ot"])
            a0, a1 = s["acc"]
            nc.vector.tensor_add(a0, a0, a1)
            nc.vector.tensor_add(a0[:, 0, :], a0[:, 0, :], a0[:, 1, :])

            def den_mm(b=b, h=h, lc2=lc2, a0=a0, s=s):
                den = psM.tile([1, 512], F32, tag="m", name=f"den{b}{h}{lc2}")
                nc.tensor.matmul(den, onesb, a0[:, 0, :], start=True, stop=True)
                fden = fdnp.tile([1, 512], F32R, tag="fd", name=f"fd{b}{h}{lc2}")
                recip_fast(fden, den)
                gc = b * 2 + lc2
                nc.vector.tensor_mul(
                    fden, fden, grow[h][:, gc * 512 : gc * 512 + 512]
                )
                s["fden"] = fden

            def bc_mm(b=b, h=h, lc2=lc2, s=s):
                bc = psM.tile([128, 512], F32, tag="m", name=f"bcd{b}{h}{lc2}")
                nc.tensor.matmul(bc, onesr, s["fden"], start=True, stop=True)
                nc.vector.tensor_mul(
                    otg[:, b, h, lc2, :], otg[:, b, h, lc2, :], bc
                )

            pend_den.append((den_mm, bc_mm))

        # one out-projection tile (2 matmuls + evac) per pair slot; the
        # serialized ypP bank is fine at this cadence
        def emit_yp(item):
            b, lc2, n = item
            li, mc = n // 4, n % 4
            if mc == 0:
                st[("ysb", b, lc2, li)] = ysbp.tile(
                    [128, 4, 512], BF16, tag="ysb", name=f"y{b}{lc2}{li}"
                )
            ysb = st[("ysb", b, lc2, li)]
            yp = ypP.tile([128, 512], F32, tag="yp", name=f"yp{b}{lc2}{n}")
            for h in range(QH):
                nc.tensor.matmul(
                    yp,
                    otg[:, b, h, lc2, li * 128 : li * 128 + 128],
                    wo_sb[:, h, mc * 512 : mc * 512 + 512],
                    start=(h == 0),
                    stop=(h == QH - 1),
                )
            if n % 2 == 0:
                nc.vector.tensor_copy(ysb[:, mc, :], yp)
            else:
                nc.scalar.copy(ysb[:, mc, :], yp)
            if mc == 3:
                row0 = b * L + lc2 * 512 + li * 128
                nc.sync.dma_start(
                    out=y[row0 : row0 + 128, :],
                    in_=ysb.rearrange("p a b -> p (a b)"),
                )

        # v transposes sprinkled into the pair stream (transpose-mode does
        # not count as PE activity for the HAM clock, so a contiguous
        # transpose stretch would re-throttle the PE to 1.2GHz)
        pend_tp = [(0, c) for c in range(NSC)] + [(1, c) for c in range(NSC)]

        def emit_tp():
            b, c = pend_tp.pop(0)
            tp = psM.tile([128, 128], BF16, tag="m", name=f"tp{b}_{c}")
            nc.tensor.transpose(
                tp, qkvt[:, 3, b * L + c * 128 : b * L + c * 128 + 128], identb
            )
            if c % 2 == 0:
                nc.vector.tensor_copy(vnew[b][:, c, :], tp)
            else:
                nc.scalar.copy(vnew[b][:, c, :], tp)

        pairs = [(bi, i) for bi in range(NB) for i in range(NP)]
        emit_sp(*pairs[0])
        emit_sp(*pairs[1])
        for g, (bi, i) in enumerate(pairs):
            b, lc2, h = blocks[bi]
            s = st[bi]
            if i == 0:
                s["ot"] = psO.tile([128, 512], F32, tag="ot", name=f"ot{bi}")
                s["acc"] = [
                    accp.tile([128, 2, 512], BF16, tag=t, name=f"ac{t}{bi}")
                    for t in ("A", "B")
                ]
            ex = exp_.tile([128, 2, 512], BF16, tag="ex", name=f"ex{bi}_{i}")
            nc.scalar.activation(ex, s["sps"].pop(i), AF.Exp)
            if bi < 2 and i < 4 and pend_tp:
                emit_tp()
                emit_tp()
            for j in range(2):
                sc = 2 * i + j
                vx = (
                    cache_tiles[b][1][:, sc, :]
                    if sc < NSC
                    else vnew[b][:, sc - NSC, :]
                )
                nc.tensor.matmul(
                    s["ot"], vx, ex[:, j, :],
                    start=(sc == 0), stop=(sc == NS - 1),
                )
            if i == 1 and pend_den:
                pend_den[0][0]()
            elif i == 3 and pend_den:
                pend_den.pop(0)[1]()
            elif i in (5, 6):
                flush_one_defer()
            if g + 2 < len(pairs):
                emit_sp(*pairs[g + 2])
            n_pop = 2 if len(pend_p3) > 20 else 1
            for _ in range(n_pop):
                if pend_p3 and pend_p3[0][3] <= g:
                    emit_yp(pend_p3.pop(0)[:3])
            tgt = s["acc"][i % 2]
            if i < 2:
                nc.vector.tensor_copy(tgt, ex)
            else:
                nc.vector.tensor_add(tgt, tgt, ex)
            if i == NP - 1:
                block_end(bi)
                if h == QH - 1:
                    g0 = (bi + 1) * NP + 5
                    pend_p3.extend(
                        (b, lc2, n, g0) for n in range(16)
                    )
        # drain: last den/bc chains, remaining out-projection tiles
        while pend_den:
            dm, bm = pend_den.pop(0)
            dm()
            bm()
        tail_pool = [None, None]  # rotate psO slabs alongside ypP in the tail

        def emit_yp_tail(item, k):
            b, lc2, n = item
            li, mc = n // 4, n % 4
            if mc == 0:
                st[("ysb", b, lc2, li)] = ysbp.tile(
                    [128, 4, 512], BF16, tag="ysb", name=f"y{b}{lc2}{li}"
                )
            ysb = st[("ysb", b, lc2, li)]
            if k % 3 == 0:
                yp = ypP.tile([128, 512], F32, tag="yp", name=f"ypt{n}")
            else:
                yp = psO.tile([128, 512], F32, tag="ot", name=f"ypt{n}")
            for h in range(QH):
                nc.tensor.matmul(
                    yp,
                    otg[:, b, h, lc2, li * 128 : li * 128 + 128],
                    wo_sb[:, h, mc * 512 : mc * 512 + 512],
                    start=(h == 0),
                    stop=(h == QH - 1),
                )
            if n % 2 == 0:
                nc.vector.tensor_copy(ysb[:, mc, :], yp)
            else:
                nc.scalar.copy(ysb[:, mc, :], yp)
            if mc == 3:
                row0 = b * L + lc2 * 512 + li * 128
                nc.sync.dma_start(
                    out=y[row0 : row0 + 128, :],
                    in_=ysb.rearrange("p a b -> p (a b)"),
                )

        for k, item in enumerate(pend_p3):
            emit_yp_tail(item[:3], k)
        pend_p3.clear()

    nc.compile()
    return nc


def _get_nc():
    global _CACHED_NC
    if _CACHED_NC is None:
        _CACHED_NC = _build_core_program()
    return _CACHED_NC


def make_in_maps(x, w_q, w_k, w_v, w_out, w_gate, cache_k, cache_v):
    bf = ml_dtypes.bfloat16
    xT = np.ascontiguousarray(x.reshape(BL, D).T)                 # [D, BL]
    xt = np.ascontiguousarray(
        xT.reshape(ND, 128, NLP, 512).transpose(1, 2, 0, 3)
    ).astype(bf)                                                  # [128,4,16,512]
    identb = np.eye(128, dtype=np.float32).astype(bf)
    onesb = np.ones((128, 1), np.float32).astype(bf)
    onesr = np.ones((1, 128), np.float32)
    in_maps = []
    for c in range(NCORES):
        g = c // 2
        wq_c = w_q[c * JC : (c + 1) * JC]                      # [256, D]
        wk_c = w_k[g * HD : (g + 1) * HD]                      # [128, D]
        wv_c = w_v[g * HD : (g + 1) * HD]
        wqkv_c = np.concatenate([wq_c, wk_c, wv_c], axis=0).T  # [D, 512]
        wqkv_c = np.ascontiguousarray(
            wqkv_c.reshape(ND, 128, 4 * HD).transpose(1, 0, 2)
        ).astype(bf)                                           # [128, 16, 512]
        wo_c = w_out[:, c * JC : (c + 1) * JC].T               # [256, D]
        wo_c = np.ascontiguousarray(
            wo_c.reshape(QH, 128, D).transpose(1, 0, 2)
        ).astype(bf)                                           # [128, 2, 2048]
        wg_c = np.ascontiguousarray(w_gate[c * QH : (c + 1) * QH].T).astype(bf)
        ckt_c = np.ascontiguousarray(cache_k[:, g].transpose(0, 2, 1)).astype(bf)
        cv_c = np.ascontiguousarray(
            cache_v[:, g].reshape(B, NSC, 128, HD).transpose(0, 2, 1, 3)
        ).astype(bf)                                           # [B,128,8,128]
        in_maps.append(
            {
                "xt": xt,
                "wqkv": wqkv_c,
                "wo": wo_c,
                "wg": wg_c,
                "ckt": ckt_c,
                "cv": cv_c,
                "identb": identb,
                "onesb": onesb,
                "onesr": onesr,
            }
        )
    return in_maps


def kernel(x, w_q, w_k, w_v, w_out, w_gate, cache_k, cache_v, _run_kwargs=None):
    in_maps = make_in_maps(x, w_q, w_k, w_v, w_out, w_gate, cache_k, cache_v)
    nc = _get_nc()
    res = run_bass_kernel_spmd(
        nc, in_maps, core_ids=list(range(NCORES)), **(_run_kwargs or {})
    )
    acc = np.zeros((BL, D), dtype=np.float32)
    for c in range(NCORES):
        acc += res.results[c]["y"].astype(np.float32)
    out = acc.reshape(B, L, D)
    if _run_kwargs:
        kernel.last_results = res
    return out


# revision 30
# speedup vs baseline: 1.0792x; 1.0792x over previous
"""Gated GQA self-attention with KV cache on 8 Trainium2 NeuronCores.

Reference computation (fp32):
    q = rms_norm((x @ w_q.T).reshape(B,L,H,HD))      # per-head rms over HD
    k = rms_norm((x @ w_k.T).reshape(B,L,HKV,HD))
    v = (x @ w_v.T).reshape(B,L,HKV,HD)
    k_t/v_t = concat(cache, new) over seq -> [B,HKV,S,HD]
    o = softmax(q @ k_t.T / sqrt(HD)) @ v_t          # full (non-causal)
    o *= sigmoid(x[..., :16] @ w_gate.T)             # per-head gate
    y = o.reshape(B,L,D) @ w_out.T

Sharding (v5): 8 cores = 4 head-groups x 2 batches.  Core c owns batch
b=c%2 and q-head group g=c//2 (4 q heads = one full GQA group, kv head g).
Unlike head-only sharding this computes each k/v projection exactly once,
and each core touches only its batch's x (half the DMA).  Per core:
q/k/v projections for 1024 tokens, attention for 4 heads over S=2048, and
two partial out-projections (one per head pair); the host sums the four
cores' partials per batch.

Engine/schedule design (carried from v4):
  * bf16 matmul operands everywhere; fp32 only in PSUM + small rows.
  * Attention = one flat software pipeline over 64 (block, s-pair) steps;
    scores for step g+2 issue before P@V of step g, across block borders.
  * exp on [128,2,512] double-chunks; softmax denominator via DVE add-tree
    + one ones-matmul per block; reciprocals via reciprocal_approx_fast.
  * gate/denominator rank-1-broadcast into the raw attention output once
    per block; head pairs share one PSUM accumulation in the out-proj.
  * out-projection spread one tile per pair slot (own PSUM bank); v
    transposes sprinkled into early pairs (transpose-mode is invisible to
    the HAM activity monitor, a contiguous stretch would cool the clock).
"""

from contextlib import ExitStack

import ml_dtypes
import numpy as np

import concourse.bass as bass  # noqa: F401
import concourse.tile as tile
from concourse import bacc, mybir
from concourse.bass_utils import run_bass_kernel_spmd

F32R = mybir.dt.float32r
F32 = mybir.dt.float32
BF16 = mybir.dt.bfloat16
AF = mybir.ActivationFunctionType

B, L, D = 2, 1024, 2048
H, HKV, HD = 16, 4, 128
CACHE = 1024
BL = B * L
S = CACHE + L               # 2048
NCORES = 8
GH = 4                      # q heads per core (one GQA group)
JCA = GH * HD               # 512 out-proj contraction cols per core
EPS = 1e-6
NLP = L // 512              # 2 l-chunks in phase 1 (own batch only)
ND = D // 128               # 16 contraction chunks
NS = S // 128               # 16 s-chunks
NSC = CACHE // 128          # 8 cached s chunks
NP = NS // 2                # 8 s-chunk pairs per block

_CACHED_NC = None


def _build_core_program():
    nc = bacc.Bacc("TRN2", target_bir_lowering=False, debug=False)

    xt = nc.dram_tensor("xt", [128, NLP, ND, 512], BF16, kind="ExternalInput").ap()
    wqkv = nc.dram_tensor("wqkv", [128, ND, 6 * HD], BF16, kind="ExternalInput").ap()
    wo = nc.dram_tensor("wo", [128, GH, D], BF16, kind="ExternalInput").ap()
    wg = nc.dram_tensor("wg", [H, GH], BF16, kind="ExternalInput").ap()
    ckt = nc.dram_tensor("ckt", [HD, CACHE], BF16, kind="ExternalInput").ap()
    cv = nc.dram_tensor("cv", [128, NSC, HD], BF16, kind="ExternalInput").ap()
    identb_in = nc.dram_tensor("identb", [128, 128], BF16, kind="ExternalInput").ap()
    onesb_in = nc.dram_tensor("onesb", [128, 1], BF16, kind="ExternalInput").ap()
    onesr_in = nc.dram_tensor("onesr", [1, 128], F32R, kind="ExternalInput").ap()
    # one partial y per head pair (their PSUM accumulations are separate)
    ya = nc.dram_tensor("ya", [L, D], BF16, kind="ExternalOutput").ap()
    yb = nc.dram_tensor("yb", [L, D], BF16, kind="ExternalOutput").ap()
    youts = [ya, yb]

    with tile.TileContext(nc) as tc, ExitStack() as ctx:
        singles = ctx.enter_context(tc.tile_pool(name="singles", bufs=1))
        xtp = ctx.enter_context(tc.tile_pool(name="xtp", bufs=2))
        sqp = ctx.enter_context(tc.tile_pool(name="sqp", bufs=2))
        exp_ = ctx.enter_context(tc.tile_pool(name="exp", bufs=6))
        accp = ctx.enter_context(tc.tile_pool(name="accp", bufs=2))
        fdnp = ctx.enter_context(tc.tile_pool(name="fdnp", bufs=2))
        ysbp = ctx.enter_context(tc.tile_pool(name="ysbp", bufs=2))

        psBig = ctx.enter_context(tc.tile_pool(name="psBig", bufs=2, space="PSUM"))
        psO = ctx.enter_context(tc.tile_pool(name="psO", bufs=2, space="PSUM"))
        psM = ctx.enter_context(tc.tile_pool(name="psM", bufs=1, space="PSUM"))
        ypP = ctx.enter_context(tc.tile_pool(name="ypP", bufs=1, space="PSUM"))

        lowp = nc.allow_low_precision(reason="bf16/f32r rounding is intended")
        ctx.enter_context(lowp)

        identb = singles.tile([128, 128], BF16)
        nc.scalar.dma_start(out=identb, in_=identb_in)
        onesb = singles.tile([128, 1], BF16)
        nc.scalar.dma_start(out=onesb, in_=onesb_in)
        onesr = singles.tile([1, 128], F32R)
        nc.scalar.dma_start(out=onesr, in_=onesr_in)
        wg_sb = singles.tile([H, GH], BF16)
        nc.scalar.dma_start(out=wg_sb, in_=wg)

        bias_q = singles.tile([1, 1], F32)
        nc.vector.memset(bias_q, HD * EPS)
        bias_k = singles.tile([1, 1], F32)
        nc.vector.memset(bias_k, EPS)

        # wqkv: k-chunk-major fine splits; first chunk on the warm scalar
        # queue so the first matmul waits on ~192KB, not megabytes
        wqkv_sb = singles.tile([128, ND, 6 * HD], BF16)
        nc.scalar.dma_start(out=wqkv_sb[:, 0:1, :], in_=wqkv[:, 0:1, :])
        for ka, kb in ((1, 4), (4, 10), (10, 16)):
            nc.sync.dma_start(
                out=wqkv_sb[:, ka:kb, :], in_=wqkv[:, ka:kb, :]
            )
        wo_sb = singles.tile([128, GH, D], BF16)

        # persistent activations, feature-on-partition; jc 0-3 = q heads,
        # 4 = k, 5 = v
        qkvt = singles.tile([128, 6, L], BF16)
        otg = singles.tile([128, 2, 2, 2, 512], BF16)   # [hp, h2, lc2]
        glog = [
            singles.tile([1, L], F32R, tag=f"glog{h}", name=f"glog{h}")
            for h in range(GH)
        ]
        grow = [
            singles.tile([1, L], F32R, tag=f"grow{h}", name=f"grow{h}")
            for h in range(GH)
        ]
        rows = [
            singles.tile([1, L], F32R, tag=f"row{i}", name=f"row{i}")
            for i in range(5)
        ]  # q0..q3, k
        cache_tiles = {}

        def emit_prefetch():
            nc.scalar.dma_start(out=wo_sb, in_=wo)
            ck_sb = singles.tile([128, CACHE], BF16, tag="ck", name="ck")
            nc.scalar.dma_start(out=ck_sb, in_=ckt)
            cv_sb = singles.tile([128, NSC, HD], BF16, tag="cv", name="cv")
            nc.scalar.dma_start(out=cv_sb, in_=cv)
            cache_tiles[0] = (ck_sb, cv_sb)
            cache_tiles["vnew"] = singles.tile(
                [128, NSC, HD], BF16, tag="vn", name="vn"
            )

        from concourse.dve_ops import (
            RECIP_APPROX_FAST_CONSTS as _RC,
            RECIPROCAL_APPROX_FAST as _RF,
        )

        def recip_fast(out_f32r, in_f32):
            nc.vector._custom_dve(
                _RF, out=out_f32r, in0=in_f32,
                s0=_RC["s0"], s1=_RC["s1"], imm2=_RC["imm2"],
            )

        pe_defer = []  # (key, fn): deferred small PE ops

        def flush_one_defer():
            if pe_defer:
                pe_defer.pop(0)[1]()

        def flush_key(key):
            for idx, (k, fn) in enumerate(pe_defer):
                if k == key:
                    pe_defer.pop(idx)
                    fn()
                    return

        def apply_norm(lc, jc):
            """reciprocal of one rms row chunk + column normalize (deferred)."""
            flush_key(("ssq", lc, jc))
            sl = slice(lc * 512, lc * 512 + 512)
            recip_fast(rows[jc][:, sl], rows[jc][:, sl].bitcast(F32))

            def bc_mul(jc=jc, sl=sl, lc=lc):
                bc = psM.tile([128, 512], F32, tag="m", name=f"bc{jc}_{lc}")
                nc.tensor.matmul(bc, onesr, rows[jc][:, sl], start=True, stop=True)
                nc.vector.tensor_mul(qkvt[:, jc, sl], qkvt[:, jc, sl], bc)

            pe_defer.append((("bc", lc, jc), bc_mul))

        # ---- phase 1: projections -------------------------------------
        for lc in range(NLP):
            sl = slice(lc * 512, lc * 512 + 512)
            xtile = xtp.tile([128, ND, 512], BF16, tag="xt")
            if lc == 0:
                nc.scalar.dma_start(out=xtile[:, 0:1, :], in_=xt[:, 0, 0:1, :])
                for ka, kb in ((1, 4), (4, 10), (10, 16)):
                    nc.sync.dma_start(
                        out=xtile[:, ka:kb, :], in_=xt[:, 0, ka:kb, :]
                    )
                emit_prefetch()
            else:
                nc.sync.dma_start(out=xtile, in_=xt[:, lc])
            for h in range(GH):
                gps = psM.tile([1, 512], F32, tag="m", name=f"gps{lc}{h}")
                nc.tensor.matmul(
                    gps, wg_sb[:, h : h + 1], xtile[0:H, 0, :],
                    start=True, stop=True,
                )
                nc.vector.tensor_copy(glog[h][:, sl], gps)
            if lc >= 1:
                apply_norm(lc - 1, 4)  # k first: attention reads it first
            for pi, pair in enumerate(((5, 4), (0, 1), (2, 3))):  # v,k first
                pp = psBig.tile([128, 2, 512], F32, tag="big", name=f"pp{lc}_{pi}")
                for j, jc in enumerate(pair):
                    for kk in range(ND):
                        nc.tensor.matmul(
                            pp[:, j, :],
                            wqkv_sb[:, kk, jc * 128 : jc * 128 + 128],
                            xtile[:, kk, :],
                            start=(kk == 0),
                            stop=(kk == ND - 1),
                        )
                    flush_one_defer()
                for j, jc in enumerate(pair):
                    if jc < GH:
                        nc.scalar.copy(qkvt[:, jc, sl], pp[:, j, :])
                    else:
                        nc.vector.tensor_copy(qkvt[:, jc, sl], pp[:, j, :])
                    if jc != 5:  # q0..q3, k: rms stats
                        sq = sqp.tile([128, 512], BF16, tag="sq")
                        nc.vector.tensor_mul(sq, qkvt[:, jc, sl], qkvt[:, jc, sl])

                        def ssq_mm(jc=jc, sl=sl, sq=sq, lc=lc):
                            ssq = psM.tile(
                                [1, 512], F32, tag="m", name=f"ssq{lc}{jc}"
                            )
                            nc.tensor.matmul(ssq, onesb, sq, start=True, stop=True)
                            scale, bias = (
                                (1.0, bias_q) if jc < GH else (1.0 / HD, bias_k)
                            )
                            nc.scalar.activation(
                                rows[jc][:, sl], ssq, AF.Sqrt,
                                bias=bias[:], scale=scale,
                            )

                        pe_defer.append((("ssq", lc, jc), ssq_mm))
                    flush_one_defer()
                if lc >= 1:
                    if pi == 0:
                        apply_norm(lc - 1, 0)
                        # gate sigmoids here: ACT is idle and the table swap
                        # stays far away from the exp stream
                        for h in range(GH):
                            nc.scalar.activation(grow[h], glog[h], AF.Sigmoid)
                    elif pi == 1:
                        apply_norm(lc - 1, 1)
                        apply_norm(lc - 1, 2)
                        apply_norm(lc, 4)  # k of the last chunk, asap
                    else:
                        apply_norm(lc - 1, 3)

        # chunk-1 q norms: reciprocals now, broadcasts drain in the early
        # attention defer slots (their consumers are ~15-45 pairs away)
        for jc in range(GH):
            apply_norm(NLP - 1, jc)

        # ---- phase 2 + 3: flat pipeline -------------------------------
        # blocks: head pair outer, then lc2, then head-in-pair
        blocks = [
            (hp, lc2, h2) for hp in range(2) for lc2 in range(2) for h2 in range(2)
        ]
        NB = len(blocks)
        st = {bi: dict() for bi in range(NB)}
        pend_den = []
        pend_p3 = []

        def q_of(bi):
            hp, lc2, h2 = blocks[bi]
            off = lc2 * 512
            return qkvt[:, 2 * hp + h2, off : off + 512]

        def emit_sp(bi, i):
            ck_sb, _ = cache_tiles[0]
            sp = psBig.tile([128, 2, 512], F32, tag="big", name=f"sp{bi}_{i}")
            for j in range(2):
                sc = 2 * i + j
                if sc < NSC:
                    kT = ck_sb[:, sc * 128 : sc * 128 + 128]
                else:
                    jj = (sc - NSC) * 128
                    kT = qkvt[:, 4, jj : jj + 128]
                nc.tensor.matmul(sp[:, j, :], kT, q_of(bi), start=True, stop=True)
            st[bi].setdefault("sps", {})[i] = sp

        def block_end(bi):
            hp, lc2, h2 = blocks[bi]
            s = st[bi]
            nc.vector.tensor_copy(otg[:, hp, h2, lc2, :], s["ot"])
            a0, a1 = s["acc"]
            nc.vector.tensor_add(a0, a0, a1)
            nc.vector.tensor_add(a0[:, 0, :], a0[:, 0, :], a0[:, 1, :])

            def den_mm(hp=hp, h2=h2, lc2=lc2, a0=a0, s=s):
                den = psM.tile([1, 512], F32, tag="m", name=f"den{hp}{h2}{lc2}")
                nc.tensor.matmul(den, onesb, a0[:, 0, :], start=True, stop=True)
                fden = fdnp.tile([1, 512], F32R, tag="fd", name=f"fd{hp}{h2}{lc2}")
                recip_fast(fden, den)
                nc.vector.tensor_mul(
                    fden, fden,
                    grow[2 * hp + h2][:, lc2 * 512 : lc2 * 512 + 512],
                )
                s["fden"] = fden

            def bc_mm(hp=hp, h2=h2, lc2=lc2, s=s):
                bc = psM.tile([128, 512], F32, tag="m", name=f"bcd{hp}{h2}{lc2}")
                nc.tensor.matmul(bc, onesr, s["fden"], start=True, stop=True)
                nc.vector.tensor_mul(
                    otg[:, hp, h2, lc2, :], otg[:, hp, h2, lc2, :], bc
                )

            pend_den.append((den_mm, bc_mm))

        def yp_mms(yp, hp, lc2, li, mc):
            for h2 in range(2):
                nc.tensor.matmul(
                    yp,
                    otg[:, hp, h2, lc2, li * 128 : li * 128 + 128],
                    wo_sb[:, 2 * hp + h2, mc * 512 : mc * 512 + 512],
                    start=(h2 == 0),
                    stop=(h2 == 1),
                )

        def yp_finish(item, yp):
            hp, lc2, n = item
            li, mc = n // 4, n % 4
            ysb = st[("ysb", hp, lc2, li)]
            if n % 2 == 0:
                nc.vector.tensor_copy(ysb[:, mc, :], yp)
            else:
                nc.scalar.copy(ysb[:, mc, :], yp)
            if mc == 3:
                row0 = lc2 * 512 + li * 128
                nc.sync.dma_start(
                    out=youts[hp][row0 : row0 + 128, :],
                    in_=ysb.rearrange("p a b -> p (a b)"),
                )

        def emit_yp(item):
            hp, lc2, n = item
            li, mc = n // 4, n % 4
            if mc == 0:
                st[("ysb", hp, lc2, li)] = ysbp.tile(
                    [128, 4, 512], BF16, tag="ysb", name=f"y{hp}{lc2}{li}"
                )
            yp = ypP.tile([128, 512], F32, tag="yp", name=f"yp{hp}{lc2}{n}")
            yp_mms(yp, hp, lc2, li, mc)
            yp_finish(item, yp)

        # v transposes sprinkled into the early pair stream
        pend_tp = list(range(NSC))

        def emit_tp():
            c = pend_tp.pop(0)
            tp = psM.tile([128, 128], BF16, tag="m", name=f"tp{c}")
            nc.tensor.transpose(
                tp, qkvt[:, 5, c * 128 : c * 128 + 128], identb
            )
            if c % 2 == 0:
                nc.vector.tensor_copy(cache_tiles["vnew"][:, c, :], tp)
            else:
                nc.scalar.copy(cache_tiles["vnew"][:, c, :], tp)

        pairs = [(bi, i) for bi in range(NB) for i in range(NP)]
        emit_sp(*pairs[0])
        emit_sp(*pairs[1])
        for g, (bi, i) in enumerate(pairs):
            hp, lc2, h2 = blocks[bi]
            s = st[bi]
            if i == 0:
                s["ot"] = psO.tile([128, 512], F32, tag="ot", name=f"ot{bi}")
                s["acc"] = [
                    accp.tile([128, 2, 512], BF16, tag=t, name=f"ac{t}{bi}")
                    for t in ("A", "B")
                ]
            ex = exp_.tile([128, 2, 512], BF16, tag="ex", name=f"ex{bi}_{i}")
            nc.scalar.activation(ex, s["sps"].pop(i), AF.Exp)
            if bi < 1 and i < 4 and pend_tp:
                emit_tp()
                emit_tp()
            for j in range(2):
                sc = 2 * i + j
                vx = (
                    cache_tiles[0][1][:, sc, :]
                    if sc < NSC
                    else cache_tiles["vnew"][:, sc - NSC, :]
                )
                nc.tensor.matmul(
                    s["ot"], vx, ex[:, j, :],
                    start=(sc == 0), stop=(sc == NS - 1),
                )
            if i == 1 and pend_den:
                pend_den[0][0]()
            elif i == 3 and pend_den:
                pend_den.pop(0)[1]()
            elif i in (5, 6):
                flush_one_defer()
            if g + 2 < len(pairs):
                emit_sp(*pairs[g + 2])
            n_pop = 2 if len(pend_p3) > 12 else 1
            for _ in range(n_pop):
                if pend_p3 and pend_p3[0][3] <= g:
                    emit_yp(pend_p3.pop(0)[:3])
            tgt = s["acc"][i % 2]
            if i < 2:
                nc.vector.tensor_copy(tgt, ex)
            else:
                nc.vector.tensor_add(tgt, tgt, ex)
            if i == NP - 1:
                block_end(bi)
                if h2 == 1:
                    g0 = (bi + 1) * NP + 5
                    pend_p3.extend((hp, lc2, n, g0) for n in range(16))
        # drain
        while pend_den:
            dm, bm = pend_den.pop(0)
            dm()
            bm()
        for k, item in enumerate(pend_p3):
            hp, lc2, n, _ = item
            li, mc = n // 4, n % 4
            if mc == 0:
                st[("ysb", hp, lc2, li)] = ysbp.tile(
                    [128, 4, 512], BF16, tag="ysb", name=f"yt{hp}{lc2}{li}"
                )
            if k % 3 == 0:
                yp = ypP.tile([128, 512], F32, tag="yp", name=f"ypt{n}")
            else:
                yp = psO.tile([128, 512], F32, tag="ot", name=f"ypt{n}")
            yp_mms(yp, hp, lc2, li, mc)
            yp_finish(item[:3], yp)
        pend_p3.clear()

    nc.compile()
    return nc


def _get_nc():
    global _CACHED_NC
    if _CACHED_NC is None:
        _CACHED_NC = _build_core_program()
    return _CACHED_NC


def make_in_maps(x, w_q, w_k, w_v, w_out, w_gate, cache_k, cache_v):
    bf = ml_dtypes.bfloat16
    identb = np.eye(128, dtype=np.float32).astype(bf)
    onesb = np.ones((128, 1), np.float32).astype(bf)
    onesr = np.ones((1, 128), np.float32)
    in_maps = []
    for c in range(NCORES):
        g, b = c // 2, c % 2
        xT = np.ascontiguousarray(x[b].T)                      # [D, L]
        xt = np.ascontiguousarray(
            xT.reshape(ND, 128, NLP, 512).transpose(1, 2, 0, 3)
        ).astype(bf)                                           # [128,2,16,512]
        wq_c = w_q[g * JCA : (g + 1) * JCA]                    # [512, D]
        wk_c = w_k[g * HD : (g + 1) * HD]                      # [128, D]
        wv_c = w_v[g * HD : (g + 1) * HD]
        wqkv_c = np.concatenate([wq_c, wk_c, wv_c], axis=0).T  # [D, 768]
        wqkv_c = np.ascontiguousarray(
            wqkv_c.reshape(ND, 128, 6 * HD).transpose(1, 0, 2)
        ).astype(bf)                                           # [128, 16, 768]
        wo_c = w_out[:, g * JCA : (g + 1) * JCA].T             # [512, D]
        wo_c = np.ascontiguousarray(
            wo_c.reshape(GH, 128, D).transpose(1, 0, 2)
        ).astype(bf)                                           # [128, 4, 2048]
        wg_c = np.ascontiguousarray(w_gate[g * GH : (g + 1) * GH].T).astype(bf)
        ckt_c = np.ascontiguousarray(cache_k[b, g].T).astype(bf)   # [128, 1024]
        cv_c = np.ascontiguousarray(
            cache_v[b, g].reshape(NSC, 128, HD).transpose(1, 0, 2)
        ).astype(bf)                                           # [128, 8, 128]
        in_maps.append(
            {
                "xt": xt,
                "wqkv": wqkv_c,
                "wo": wo_c,
                "wg": wg_c,
                "ckt": ckt_c,
                "cv": cv_c,
                "identb": identb,
                "onesb": onesb,
                "onesr": onesr,
            }
        )
    return in_maps


def kernel(x, w_q, w_k, w_v, w_out, w_gate, cache_k, cache_v, _run_kwargs=None):
    in_maps = make_in_maps(x, w_q, w_k, w_v, w_out, w_gate, cache_k, cache_v)
    nc = _get_nc()
    res = run_bass_kernel_spmd(
        nc, in_maps, core_ids=list(range(NCORES)), **(_run_kwargs or {})
    )
    out = np.zeros((B, L, D), dtype=np.float32)
    for c in range(NCORES):
        b = c % 2
        out[b] += res.results[c]["ya"].astype(np.float32)
        out[b] += res.results[c]["yb"].astype(np.float32)
    if _run_kwargs:
        kernel.last_results = res
    return out
